# revision 18
# baseline (speedup 1.0000x reference)
"""Trainium2 Bass kernel for nn_BasisFunction2D (2-D basis-function embedding lookup).

Reformulation: the data-dependent bilinear interpolation over a 16x16 grid of
per-(ix,iz) tables is expressed as dense "hat-function" interpolation matrices

    V[(q,iz), b] = hat_q(z[iz,b])      (z-side weights, 2 nonzeros per column)
    U[b, (p,ix)] = hat_p(x[ix,b])      (x-side weights, transposed layout)

so that   out[o,b] = sum_{k,m} V[k,b] * G_o[k,m] * U[b,m]
with      G_o[(q,iz),(p,ix)] = func_parameter[p,q,o,ix,iz].

Per core (output dim o sharded 8-ways, 8 o's per core):
  stage 1 (TensorE, fp32r):  C_o[b, m] = sum_k V[k,b] G_o[k,m]   (PSUM, K tiled by 128)
  stage 2 (VectorE):         out[b,o] = sum_m C_o[b,m] * U[b,m]  (fused mult+reduce)

The hat functions (including the reference's linear tail extrapolation) are built
on-device from affine+relu+min ops; host only reshapes/replicates inputs.
"""

import numpy as np

import concourse.bass as bass
import concourse.bacc as bacc_mod
import concourse.tile as tile
from concourse import mybir
from concourse.bass_utils import run_bass_kernel_spmd

F32 = mybir.dt.float32
F32R = mybir.dt.float32r
AF = mybir.ActivationFunctionType
ALU = mybir.AluOpType

NCORES = 8
NG = 16            # grid bins
NQ = 17            # grid corners per axis
IX = 32
IZ = 32
OUT = 64
B = 512
OSH = OUT // NCORES          # outputs per core = 8
K = NQ * IZ                  # 544 contraction rows (q,iz)
M = NQ * IX                  # 544 free cols (p,ix)
NSPL = 272                   # N split halves (both >=256 for fp32r full rate)
BIG = 1e30
KCH = [(0, 128), (128, 128), (256, 128), (384, 128), (512, 32)]
NBC = B // 128               # 4 batch chunks

_NC_CACHE = {}


def _build_nc(bd, il, debug=False):
    """Build the single-core Bass/Tile program (identical across cores)."""
    bd = [float(v) for v in bd]
    il = [float(v) for v in il]

    nc = bacc_mod.Bacc(None, target_bir_lowering=False)
    gt_d = nc.dram_tensor("gt", [OSH, K, M], F32R, kind="ExternalInput")
    zv_d = nc.dram_tensor("zv", [K, B + 8], F32, kind="ExternalInput")
    zrpad_d = nc.dram_tensor("zrpad", [B, M + 32], F32, kind="ExternalInput")
    out_d = nc.dram_tensor("out", [B, OSH], F32, kind="ExternalOutput")
    if debug:
        dbgv_d = nc.dram_tensor("dbg_v", [K, B], F32, kind="ExternalOutput")
        dbgu_d = nc.dram_tensor("dbg_u", [B, M], F32, kind="ExternalOutput")
        dbgc_d = nc.dram_tensor("dbg_c", [B, M], F32, kind="ExternalOutput")

    with tile.TileContext(nc) as tc:
        with (
            tc.tile_pool(name="per", bufs=1) as per,       # persistent tiles
            tc.tile_pool(name="tmp", bufs=1) as tmp,       # V/U build scratch
            tc.tile_pool(name="sc", bufs=4) as sc,         # ttr mandatory outs
            tc.tile_pool(name="ac", bufs=4) as ac,         # [128,1] accumulators
            tc.tile_pool(name="ps", bufs=8, space="PSUM") as ps,
        ):
            # ---------------- input loads ----------------
            zv_sb, V_sb = [], []
            for kci, (r0, rows) in enumerate(KCH):
                zt = per.tile([rows, B + 8], F32, tag=f"zv{kci}", name=f"zv{kci}")
                nc.sync.dma_start(zt[:], zv_d[r0:r0 + rows, :])
                zv_sb.append(zt)
                V_sb.append(per.tile([rows, B], F32R, tag=f"V{kci}", name=f"V{kci}"))
            zrep_sb = [t[:, 0:B] for t in zv_sb]
            vcoef_sb = [t[:, B:B + 8] for t in zv_sb]

            zrpad_sb, U_sb, outT_sb = [], [], []
            for bc in range(NBC):
                pt = per.tile([128, M + 32], F32, tag=f"zrpad{bc}", name=f"zrpad{bc}")
                nc.sync.dma_start(pt[:], zrpad_d[bc * 128:(bc + 1) * 128, :])
                zrpad_sb.append(pt)
                U_sb.append(per.tile([128, M], F32, tag=f"U{bc}", name=f"U{bc}"))
                outT_sb.append(per.tile([128, OSH], F32, tag=f"outT{bc}", name=f"outT{bc}"))

            G_sb = []
            for o in range(OSH):
                row = []
                for kci, (r0, rows) in enumerate(KCH):
                    gtile = per.tile([rows, M], F32R, tag=f"G{o}_{kci}", name=f"G{o}_{kci}")
                    nc.sync.dma_start(gtile[:], gt_d[o, r0:r0 + rows, :])
                    row.append(gtile)
                G_sb.append(row)

            # ---------------- build V[(q,iz), b] ----------------
            # rows r = q*32 + iz;  L = (z - bd[q-1])*il[q-1], R = (bd[q+1] - z)*il[q]
            # V = min(relu(L), relu(R)); per-row (scale,bias) from vcoef cols.
            for kci, (r0, rows) in enumerate(KCH):
                lp = tmp.tile([rows, B], F32, tag=f"vL{kci}", name=f"vL{kci}")
                nc.scalar.activation(lp[:], zrep_sb[kci], AF.Relu,
                                     bias=vcoef_sb[kci][:, 1:2], scale=vcoef_sb[kci][:, 0:1])
                rp = tmp.tile([rows, B], F32, tag=f"vR{kci}", name=f"vR{kci}")
                nc.scalar.activation(rp[:], zrep_sb[kci], AF.Relu,
                                     bias=vcoef_sb[kci][:, 3:4], scale=vcoef_sb[kci][:, 2:3])
                nc.vector.tensor_tensor(V_sb[kci][:], lp[:], rp[:], ALU.min)

            # tail fixes: q=1 keeps L un-relu'd (left extrapolation), q=15 keeps R raw.
            fl = tmp.tile([32, B], F32, tag="vfq1L", name="vfixq1L")
            nc.scalar.activation(fl[:], zv_sb[0][32:64, 0:B], AF.Copy,
                                 bias=float(-bd[0] * il[0]), scale=float(il[0]))
            fr = tmp.tile([32, B], F32, tag="vfq1R", name="vfixq1R")
            nc.scalar.activation(fr[:], zv_sb[0][32:64, 0:B], AF.Relu,
                                 bias=zv_sb[0][32:64, B + 4:B + 5], scale=float(-il[1]))
            nc.vector.tensor_tensor(V_sb[0][32:64, :], fl[:], fr[:], ALU.min)

            fl2 = tmp.tile([32, B], F32, tag="vfq15L", name="vfixq15L")
            nc.scalar.activation(fl2[:], zv_sb[3][96:128, 0:B], AF.Relu,
                                 bias=zv_sb[3][96:128, B + 4:B + 5], scale=float(il[14]))
            fr2 = tmp.tile([32, B], F32, tag="vfq15R", name="vfixq15R")
            nc.scalar.activation(fr2[:], zv_sb[3][96:128, 0:B], AF.Copy,
                                 bias=float(bd[16] * il[15]), scale=float(-il[15]))
            nc.vector.tensor_tensor(V_sb[3][96:128, :], fl2[:], fr2[:], ALU.min)

            # ---------------- build U[b, (p,ix)] (transposed layout) ----------------
            # zrpad col 32+p*32+ix = R_p(x[ix,b]); identity L_p = 1 - R_{p-1} gives the
            # L operand as a 32-col-shifted read of the same tensor.
            for bc in range(NBC):
                lp = tmp.tile([128, M], F32, tag=f"uL{bc}", name=f"uL{bc}")
                nc.scalar.activation(lp[:], zrpad_sb[bc][:, 0:M], AF.Relu,
                                     bias=1.0, scale=-1.0)
                rp = tmp.tile([128, M], F32, tag=f"uR{bc}", name=f"uR{bc}")
                nc.scalar.activation(rp[:], zrpad_sb[bc][:, 32:M + 32], AF.Relu)
                nc.vector.tensor_tensor(U_sb[bc][:], lp[:], rp[:], ALU.min)
                # p=1 fix: L un-relu'd
                fx = tmp.tile([128, 32], F32, tag=f"ufix{bc}", name=f"ufix{bc}")
                nc.scalar.activation(fx[:], zrpad_sb[bc][:, 32:64], AF.Copy,
                                     bias=1.0, scale=-1.0)
                nc.vector.tensor_tensor(U_sb[bc][:, 32:64], fx[:], rp[:, 32:64], ALU.min)
                # p=15 fix: R raw (direct from input)
                nc.vector.tensor_tensor(U_sb[bc][:, 480:512], lp[:, 480:512],
                                        zrpad_sb[bc][:, 512:544], ALU.min)

            if debug:
                for kci, (r0, rows) in enumerate(KCH):
                    nc.sync.dma_start(dbgv_d[r0:r0 + rows, :], V_sb[kci][:])
                for bc in range(NBC):
                    nc.sync.dma_start(dbgu_d[bc * 128:(bc + 1) * 128, :], U_sb[bc][:])

            # ---------------- main loop: matmuls + fused reduce ----------------
            for o in range(OSH):
                for bc in range(NBC):
                    psA = ps.tile([128, NSPL], F32, tag="ps", name=f"psA{o}_{bc}")
                    psB = ps.tile([128, NSPL], F32, tag="ps", name=f"psB{o}_{bc}")
                    for kci, (r0, rows) in enumerate(KCH):
                        lhs = V_sb[kci][0:rows, bc * 128:(bc + 1) * 128]
                        st, sp = (kci == 0), (kci == len(KCH) - 1)
                        nc.tensor.matmul(psA[:], lhs,
                                         G_sb[o][kci][0:rows, 0:NSPL],
                                         start=st, stop=sp)
                        nc.tensor.matmul(psB[:], lhs,
                                         G_sb[o][kci][0:rows, NSPL:M],
                                         start=st, stop=sp)
                    if debug and o == 0:
                        dtmpA = sc.tile([128, NSPL], F32, tag="dbg", name=f"dbgA{bc}")
                        nc.vector.tensor_copy(dtmpA[:], psA[:])
                        nc.sync.dma_start(dbgc_d[bc * 128:(bc + 1) * 128, 0:NSPL], dtmpA[:])
                        dtmpB = sc.tile([128, NSPL], F32, tag="dbg", name=f"dbgB{bc}")
                        nc.vector.tensor_copy(dtmpB[:], psB[:])
                        nc.sync.dma_start(dbgc_d[bc * 128:(bc + 1) * 128, NSPL:M], dtmpB[:])
                    scA = sc.tile([128, NSPL], F32, tag="sc", name=f"scA{o}_{bc}")
                    acc1 = ac.tile([128, 1], F32, tag="ac", name=f"acc1_{o}_{bc}")
                    nc.vector.scalar_tensor_tensor(
                        out=scA[:], in0=psA[:], scalar=1.0, in1=U_sb[bc][:, 0:NSPL],
                        op0=ALU.mult, op1=ALU.mult, accum_out=acc1[:])
                    scB = sc.tile([128, NSPL], F32, tag="sc", name=f"scB{o}_{bc}")
                    acc2 = ac.tile([128, 1], F32, tag="ac2", name=f"acc2_{o}_{bc}")
                    nc.vector.scalar_tensor_tensor(
                        out=scB[:], in0=psB[:], scalar=1.0, in1=U_sb[bc][:, NSPL:M],
                        op0=ALU.mult, op1=ALU.mult, accum_out=acc2[:])
                    nc.vector.tensor_add(outT_sb[bc][:, o:o + 1], acc1[:], acc2[:])

            # ---------------- store ----------------
            for bc in range(NBC):
                nc.sync.dma_start(out_d[bc * 128:(bc + 1) * 128, :], outT_sb[bc][:])

    nc.finalize()
    return nc


def _host_prep(x, z, func_parameter, borders, il):
    x = np.asarray(x, np.float32)
    z = np.asarray(z, np.float32)
    F = np.asarray(func_parameter, np.float32)
    bd = np.asarray(borders, np.float32)
    il = np.asarray(il, np.float32)

    # G_all[o, q*32+iz, p*32+ix] = F[p,q,o,ix,iz]
    G_all = np.ascontiguousarray(F.transpose(2, 1, 4, 0, 3)).reshape(OUT, K, M)

    zrep = np.tile(z, (NQ, 1))                                # [544, 512]

    q = np.arange(NQ)
    aL = np.where(q >= 1, il[np.clip(q - 1, 0, NG - 1)], 0.0)
    bL = np.where(q >= 1, -bd[np.clip(q - 1, 0, NQ - 1)] * il[np.clip(q - 1, 0, NG - 1)], BIG)
    aR = np.where(q <= NG - 1, -il[np.clip(q, 0, NG - 1)], 0.0)
    bR = np.where(q <= NG - 1, bd[np.clip(q + 1, 0, NQ - 1)] * il[np.clip(q, 0, NG - 1)], BIG)
    fixb = np.zeros(NQ)
    fixb[1] = bd[2] * il[1]          # V q=1 fix: R-side relu bias
    fixb[15] = -bd[14] * il[14]      # V q=15 fix: L-side relu bias
    zero = np.zeros(NQ)
    vcoef = np.ascontiguousarray(
        np.stack([np.repeat(c.astype(np.float32), IZ)
                  for c in (aL, bL, aR, bR, fixb, zero, zero, zero)], axis=1))

    Rx = np.empty((NQ, IX, B), np.float32)
    for p in range(NG):
        Rx[p] = (bd[p + 1] - x) * il[p]
    Rx[NG] = BIG
    ZR_T = Rx.transpose(2, 0, 1).reshape(B, M)
    zrpad = np.ascontiguousarray(
        np.concatenate([np.full((B, 32), -BIG, np.float32), ZR_T], axis=1))

    zv = np.ascontiguousarray(np.concatenate([zrep, vcoef], axis=1))  # [544, 520]
    return G_all, zv, zrpad, bd, il


def kernel(x, z, func_parameter, borders, inverse_chunk_lengths, _trace=False):
    G_all, zv, zrpad, bd, il = _host_prep(
        x, z, func_parameter, borders, inverse_chunk_lengths)

    key = (bd.tobytes(), il.tobytes())
    if key not in _NC_CACHE:
        _NC_CACHE[key] = _build_nc(bd, il)
    nc = _NC_CACHE[key]

    in_maps = []
    for c in range(NCORES):
        in_maps.append({
            "gt": np.ascontiguousarray(G_all[c * OSH:(c + 1) * OSH]),
            "zv": zv,
            "zrpad": zrpad,
        })

    res = run_bass_kernel_spmd(nc, in_maps, core_ids=list(range(NCORES)),
                               trace=_trace)
    out = np.concatenate([res.results[c]["out"].T for c in range(NCORES)], axis=0)
    out = np.ascontiguousarray(out.astype(np.float32))
    if _trace:
        return out, res
    return out


# revision 20
# speedup vs baseline: 1.0066x; 1.0066x over previous
"""Trainium2 Bass kernel for nn_BasisFunction2D (2-D basis-function embedding lookup).

Reformulation: the data-dependent bilinear interpolation over a 16x16 grid of
per-(ix,iz) tables is expressed as dense "hat-function" interpolation matrices

    V[(q,iz), b] = hat_q(z[iz,b])      (z-side weights, 2 nonzeros per column)
    U[b, (p,ix)] = hat_p(x[ix,b])      (x-side weights, transposed layout)

so that   out[o,b] = sum_{k,m} V[k,b] * G_o[k,m] * U[b,m]
with      G_o[(q,iz),(p,ix)] = func_parameter[p,q,o,ix,iz].

Per core (output dim o sharded 8-ways, 8 o's per core):
  stage 1 (TensorE, fp32r):  C_o[b, m] = sum_k V[k,b] G_o[k,m]   (PSUM, K tiled by 128)
  stage 2 (VectorE):         out[b,o] = sum_m C_o[b,m] * U[b,m]  (fused mult+reduce)

The hat functions (including the reference's linear tail extrapolation) are built
on-device from affine+relu+min ops; host only reshapes/replicates inputs.
"""

import numpy as np

import concourse.bass as bass
import concourse.bacc as bacc_mod
import concourse.tile as tile
from concourse import mybir
from concourse.bass_utils import run_bass_kernel_spmd

F32 = mybir.dt.float32
F32R = mybir.dt.float32r
AF = mybir.ActivationFunctionType
ALU = mybir.AluOpType

NCORES = 8
NG = 16            # grid bins
NQ = 17            # grid corners per axis
IX = 32
IZ = 32
OUT = 64
B = 512
OSH = OUT // NCORES          # outputs per core = 8
K = NQ * IZ                  # 544 contraction rows (q,iz)
M = NQ * IX                  # 544 free cols (p,ix)
NSPL = 272                   # N split halves (both >=256 for fp32r full rate)
BIG = 1e30
KCH = [(0, 128), (128, 128), (256, 128), (384, 128), (512, 32)]
NBC = B // 128               # 4 batch chunks

_NC_CACHE = {}


def _build_nc(bd, il, debug=False):
    """Build the single-core Bass/Tile program (identical across cores)."""
    bd = [float(v) for v in bd]
    il = [float(v) for v in il]

    nc = bacc_mod.Bacc(None, target_bir_lowering=False)
    gt_d = nc.dram_tensor("gt", [OSH, K, M], F32R, kind="ExternalInput")
    zv_d = nc.dram_tensor("zv", [K, B + 8], F32, kind="ExternalInput")
    zrpad_d = nc.dram_tensor("zrpad", [B, M + 32], F32, kind="ExternalInput")
    out_d = nc.dram_tensor("out", [B, OSH], F32, kind="ExternalOutput")
    if debug:
        dbgv_d = nc.dram_tensor("dbg_v", [K, B], F32, kind="ExternalOutput")
        dbgu_d = nc.dram_tensor("dbg_u", [B, M], F32, kind="ExternalOutput")
        dbgc_d = nc.dram_tensor("dbg_c", [B, M], F32, kind="ExternalOutput")

    with tile.TileContext(nc) as tc:
        with (
            tc.tile_pool(name="per", bufs=1) as per,       # persistent tiles
            tc.tile_pool(name="tmp", bufs=1) as tmp,       # V/U build scratch
            tc.tile_pool(name="sc", bufs=6) as sc,         # stt mandatory outs
            tc.tile_pool(name="ac", bufs=6) as ac,         # [128,1] accumulators
            tc.tile_pool(name="ps", bufs=8, space="PSUM") as ps,
        ):
            # PE warmup: dependency-free dummy matmuls overlap the DMA phase and
            # flip the HAM clock-gate to 8/8 before the real matmuls start.
            wt = per.tile([128, B], mybir.dt.bfloat16, tag="warm", name="wt")
            nc.vector.memset(wt[:], 0.0)
            wps = ps.tile([128, B], F32, tag="ps", name="wps")
            for _ in range(10):
                nc.tensor.matmul(wps[:], wt[:, 0:128], wt[:], start=True, stop=True)

            # ---------------- input loads ----------------
            zv_sb, V_sb = [], []
            for kci, (r0, rows) in enumerate(KCH):
                zt = per.tile([rows, B + 8], F32, tag=f"zv{kci}", name=f"zv{kci}")
                nc.sync.dma_start(zt[:], zv_d[r0:r0 + rows, :])
                zv_sb.append(zt)
                V_sb.append(per.tile([rows, B], F32R, tag=f"V{kci}", name=f"V{kci}"))
            zrep_sb = [t[:, 0:B] for t in zv_sb]
            vcoef_sb = [t[:, B:B + 8] for t in zv_sb]

            zrpad_sb, U_sb, outT_sb = [], [], []
            for bc in range(NBC):
                pt = per.tile([128, M + 32], F32, tag=f"zrpad{bc}", name=f"zrpad{bc}")
                nc.sync.dma_start(pt[:], zrpad_d[bc * 128:(bc + 1) * 128, :])
                zrpad_sb.append(pt)
                U_sb.append(per.tile([128, M], F32, tag=f"U{bc}", name=f"U{bc}"))
                outT_sb.append(per.tile([128, OSH], F32, tag=f"outT{bc}", name=f"outT{bc}"))

            G_sb = []
            for o in range(OSH):
                row = []
                for kci, (r0, rows) in enumerate(KCH):
                    gtile = per.tile([rows, M], F32R, tag=f"G{o}_{kci}", name=f"G{o}_{kci}")
                    nc.sync.dma_start(gtile[:], gt_d[o, r0:r0 + rows, :])
                    row.append(gtile)
                G_sb.append(row)

            # ---------------- build V[(q,iz), b] ----------------
            # rows r = q*32 + iz;  L = (z - bd[q-1])*il[q-1], R = (bd[q+1] - z)*il[q]
            # V = min(relu(L), relu(R)); per-row (scale,bias) from vcoef cols.
            for kci, (r0, rows) in enumerate(KCH):
                lp = tmp.tile([rows, B], F32, tag=f"vL{kci}", name=f"vL{kci}")
                nc.scalar.activation(lp[:], zrep_sb[kci], AF.Relu,
                                     bias=vcoef_sb[kci][:, 1:2], scale=vcoef_sb[kci][:, 0:1])
                rp = tmp.tile([rows, B], F32, tag=f"vR{kci}", name=f"vR{kci}")
                nc.scalar.activation(rp[:], zrep_sb[kci], AF.Relu,
                                     bias=vcoef_sb[kci][:, 3:4], scale=vcoef_sb[kci][:, 2:3])
                nc.vector.tensor_tensor(V_sb[kci][:], lp[:], rp[:], ALU.min)

            # tail fixes: q=1 keeps L un-relu'd (left extrapolation), q=15 keeps R raw.
            fl = tmp.tile([32, B], F32, tag="vfq1L", name="vfixq1L")
            nc.scalar.activation(fl[:], zv_sb[0][32:64, 0:B], AF.Copy,
                                 bias=float(-bd[0] * il[0]), scale=float(il[0]))
            fr = tmp.tile([32, B], F32, tag="vfq1R", name="vfixq1R")
            nc.scalar.activation(fr[:], zv_sb[0][32:64, 0:B], AF.Relu,
                                 bias=zv_sb[0][32:64, B + 4:B + 5], scale=float(-il[1]))
            nc.vector.tensor_tensor(V_sb[0][32:64, :], fl[:], fr[:], ALU.min)

            fl2 = tmp.tile([32, B], F32, tag="vfq15L", name="vfixq15L")
            nc.scalar.activation(fl2[:], zv_sb[3][96:128, 0:B], AF.Relu,
                                 bias=zv_sb[3][96:128, B + 4:B + 5], scale=float(il[14]))
            fr2 = tmp.tile([32, B], F32, tag="vfq15R", name="vfixq15R")
            nc.scalar.activation(fr2[:], zv_sb[3][96:128, 0:B], AF.Copy,
                                 bias=float(bd[16] * il[15]), scale=float(-il[15]))
            nc.vector.tensor_tensor(V_sb[3][96:128, :], fl2[:], fr2[:], ALU.min)

            # ---------------- build U[b, (p,ix)] (transposed layout) ----------------
            # zrpad col 32+p*32+ix = R_p(x[ix,b]); identity L_p = 1 - R_{p-1} gives the
            # L operand as a 32-col-shifted read of the same tensor.
            for bc in range(NBC):
                lp = tmp.tile([128, M], F32, tag=f"uL{bc}", name=f"uL{bc}")
                nc.scalar.activation(lp[:], zrpad_sb[bc][:, 0:M], AF.Relu,
                                     bias=1.0, scale=-1.0)
                rp = tmp.tile([128, M], F32, tag=f"uR{bc}", name=f"uR{bc}")
                nc.scalar.activation(rp[:], zrpad_sb[bc][:, 32:M + 32], AF.Relu)
                nc.vector.tensor_tensor(U_sb[bc][:], lp[:], rp[:], ALU.min)
                # p=1 fix: L un-relu'd
                fx = tmp.tile([128, 32], F32, tag=f"ufix{bc}", name=f"ufix{bc}")
                nc.scalar.activation(fx[:], zrpad_sb[bc][:, 32:64], AF.Copy,
                                     bias=1.0, scale=-1.0)
                nc.vector.tensor_tensor(U_sb[bc][:, 32:64], fx[:], rp[:, 32:64], ALU.min)
                # p=15 fix: R raw (direct from input)
                nc.vector.tensor_tensor(U_sb[bc][:, 480:512], lp[:, 480:512],
                                        zrpad_sb[bc][:, 512:544], ALU.min)

            if debug:
                for kci, (r0, rows) in enumerate(KCH):
                    nc.sync.dma_start(dbgv_d[r0:r0 + rows, :], V_sb[kci][:])
                for bc in range(NBC):
                    nc.sync.dma_start(dbgu_d[bc * 128:(bc + 1) * 128, :], U_sb[bc][:])

            # ---------------- main loop: matmuls + fused reduce ----------------
            for o in range(OSH):
                for bc in range(NBC):
                    psA = ps.tile([128, NSPL], F32, tag="ps", name=f"psA{o}_{bc}")
                    psB = ps.tile([128, NSPL], F32, tag="ps", name=f"psB{o}_{bc}")
                    for kci, (r0, rows) in enumerate(KCH):
                        lhs = V_sb[kci][0:rows, bc * 128:(bc + 1) * 128]
                        st, sp = (kci == 0), (kci == len(KCH) - 1)
                        nc.tensor.matmul(psA[:], lhs,
                                         G_sb[o][kci][0:rows, 0:NSPL],
                                         start=st, stop=sp)
                        nc.tensor.matmul(psB[:], lhs,
                                         G_sb[o][kci][0:rows, NSPL:M],
                                         start=st, stop=sp)
                    if debug and o == 0:
                        dtmpA = sc.tile([128, NSPL], F32, tag="dbg", name=f"dbgA{bc}")
                        nc.vector.tensor_copy(dtmpA[:], psA[:])
                        nc.sync.dma_start(dbgc_d[bc * 128:(bc + 1) * 128, 0:NSPL], dtmpA[:])
                        dtmpB = sc.tile([128, NSPL], F32, tag="dbg", name=f"dbgB{bc}")
                        nc.vector.tensor_copy(dtmpB[:], psB[:])
                        nc.sync.dma_start(dbgc_d[bc * 128:(bc + 1) * 128, NSPL:M], dtmpB[:])
                    scA = sc.tile([128, NSPL], F32, tag="sc", name=f"scA{o}_{bc}")
                    acc1 = ac.tile([128, 1], F32, tag="ac", name=f"acc1_{o}_{bc}")
                    nc.vector.scalar_tensor_tensor(
                        out=scA[:], in0=psA[:], scalar=1.0, in1=U_sb[bc][:, 0:NSPL],
                        op0=ALU.mult, op1=ALU.mult, accum_out=acc1[:])
                    scB = sc.tile([128, NSPL], F32, tag="sc", name=f"scB{o}_{bc}")
                    acc2 = ac.tile([128, 1], F32, tag="ac2", name=f"acc2_{o}_{bc}")
                    nc.vector.scalar_tensor_tensor(
                        out=scB[:], in0=psB[:], scalar=1.0, in1=U_sb[bc][:, NSPL:M],
                        op0=ALU.mult, op1=ALU.mult, accum_out=acc2[:])
                    nc.vector.tensor_add(outT_sb[bc][:, o:o + 1], acc1[:], acc2[:])

            # ---------------- store ----------------
            for bc in range(NBC):
                nc.sync.dma_start(out_d[bc * 128:(bc + 1) * 128, :], outT_sb[bc][:])

    nc.finalize()
    return nc


def _host_prep(x, z, func_parameter, borders, il):
    x = np.asarray(x, np.float32)
    z = np.asarray(z, np.float32)
    F = np.asarray(func_parameter, np.float32)
    bd = np.asarray(borders, np.float32)
    il = np.asarray(il, np.float32)

    # G_all[o, q*32+iz, p*32+ix] = F[p,q,o,ix,iz]
    G_all = np.ascontiguousarray(F.transpose(2, 1, 4, 0, 3)).reshape(OUT, K, M)

    zrep = np.tile(z, (NQ, 1))                                # [544, 512]

    q = np.arange(NQ)
    aL = np.where(q >= 1, il[np.clip(q - 1, 0, NG - 1)], 0.0)
    bL = np.where(q >= 1, -bd[np.clip(q - 1, 0, NQ - 1)] * il[np.clip(q - 1, 0, NG - 1)], BIG)
    aR = np.where(q <= NG - 1, -il[np.clip(q, 0, NG - 1)], 0.0)
    bR = np.where(q <= NG - 1, bd[np.clip(q + 1, 0, NQ - 1)] * il[np.clip(q, 0, NG - 1)], BIG)
    fixb = np.zeros(NQ)
    fixb[1] = bd[2] * il[1]          # V q=1 fix: R-side relu bias
    fixb[15] = -bd[14] * il[14]      # V q=15 fix: L-side relu bias
    zero = np.zeros(NQ)
    vcoef = np.ascontiguousarray(
        np.stack([np.repeat(c.astype(np.float32), IZ)
                  for c in (aL, bL, aR, bR, fixb, zero, zero, zero)], axis=1))

    Rx = np.empty((NQ, IX, B), np.float32)
    for p in range(NG):
        Rx[p] = (bd[p + 1] - x) * il[p]
    Rx[NG] = BIG
    ZR_T = Rx.transpose(2, 0, 1).reshape(B, M)
    zrpad = np.ascontiguousarray(
        np.concatenate([np.full((B, 32), -BIG, np.float32), ZR_T], axis=1))

    zv = np.ascontiguousarray(np.concatenate([zrep, vcoef], axis=1))  # [544, 520]
    return G_all, zv, zrpad, bd, il


def kernel(x, z, func_parameter, borders, inverse_chunk_lengths, _trace=False):
    G_all, zv, zrpad, bd, il = _host_prep(
        x, z, func_parameter, borders, inverse_chunk_lengths)

    key = (bd.tobytes(), il.tobytes())
    if key not in _NC_CACHE:
        _NC_CACHE[key] = _build_nc(bd, il)
    nc = _NC_CACHE[key]

    in_maps = []
    for c in range(NCORES):
        in_maps.append({
            "gt": np.ascontiguousarray(G_all[c * OSH:(c + 1) * OSH]),
            "zv": zv,
            "zrpad": zrpad,
        })

    res = run_bass_kernel_spmd(nc, in_maps, core_ids=list(range(NCORES)),
                               trace=_trace)
    out = np.concatenate([res.results[c]["out"].T for c in range(NCORES)], axis=0)
    out = np.ascontiguousarray(out.astype(np.float32))
    if _trace:
        return out, res
    return out


# revision 21
# speedup vs baseline: 1.6269x; 1.6161x over previous
"""Trainium2 Bass kernel for nn_BasisFunction2D (2-D basis-function embedding lookup).

Reformulation: the data-dependent bilinear interpolation over a 16x16 grid of
per-(ix,iz) tables is expressed as dense "hat-function" interpolation matrices

    V[(q,iz), b] = hat_q(z[iz,b])      (z-side weights, 2 nonzeros per column)
    U[b, (p,ix)] = hat_p(x[ix,b])      (x-side weights, transposed layout)

so that   out[o,b] = sum_{k,m} V[k,b] * G_o[k,m] * U[b,m]
with      G_o[(q,iz),(p,ix)] = func_parameter[p,q,o,ix,iz].

Per core (output dim o sharded 8-ways, 8 o's per core):
  stage 1 (TensorE, fp32r):  C_o[b, m] = sum_k V[k,b] G_o[k,m]   (PSUM, K tiled by 128)
  stage 2 (VectorE):         out[b,o] = sum_m C_o[b,m] * U[b,m]  (fused mult+reduce)

The hat functions (including the reference's linear tail extrapolation) are built
on-device from affine+relu+min ops; host only reshapes/replicates inputs.
"""

import numpy as np

import concourse.bass as bass
import concourse.bacc as bacc_mod
import concourse.tile as tile
from concourse import mybir
from concourse.bass_utils import run_bass_kernel_spmd

F32 = mybir.dt.float32
F32R = mybir.dt.float32r
BF16 = mybir.dt.bfloat16
AF = mybir.ActivationFunctionType
ALU = mybir.AluOpType

NCORES = 8
NG = 16            # grid bins
NQ = 17            # grid corners per axis
IX = 32
IZ = 32
OUT = 64
B = 512
OSH = OUT // NCORES          # outputs per core = 8
K = NQ * IZ                  # 544 contraction rows (q,iz)
M = NQ * IX                  # 544 free cols (p,ix)
NSPL = 272                   # N split halves (both >=256 for fp32r full rate)
BIG = 1e30
KCH = [(0, 128), (128, 128), (256, 128), (384, 128), (512, 32)]
NBC = B // 128               # 4 batch chunks

_NC_CACHE = {}


def _build_nc(bd, il, debug=False):
    """Build the single-core Bass/Tile program (identical across cores)."""
    bd = [float(v) for v in bd]
    il = [float(v) for v in il]

    nc = bacc_mod.Bacc(None, target_bir_lowering=False)
    gt_d = nc.dram_tensor("gt", [OSH, K, M], BF16, kind="ExternalInput")
    zv_d = nc.dram_tensor("zv", [K, B + 8], F32, kind="ExternalInput")
    zrpad_d = nc.dram_tensor("zrpad", [B, M + 32], F32, kind="ExternalInput")
    out_d = nc.dram_tensor("out", [B, OSH], F32, kind="ExternalOutput")
    if debug:
        dbgv_d = nc.dram_tensor("dbg_v", [K, B], F32, kind="ExternalOutput")
        dbgu_d = nc.dram_tensor("dbg_u", [B, M], F32, kind="ExternalOutput")
        dbgc_d = nc.dram_tensor("dbg_c", [B, M], F32, kind="ExternalOutput")

    with tile.TileContext(nc) as tc:
        with (
            tc.tile_pool(name="per", bufs=1) as per,       # persistent tiles
            tc.tile_pool(name="tmp", bufs=1) as tmp,       # V/U build scratch
            tc.tile_pool(name="sc", bufs=6) as sc,         # stt mandatory outs
            tc.tile_pool(name="ac", bufs=6) as ac,         # [128,1] accumulators
            tc.tile_pool(name="ps", bufs=8, space="PSUM") as ps,
        ):
            # PE warmup: dependency-free dummy matmuls overlap the DMA phase and
            # flip the HAM clock-gate to 8/8 before the real matmuls start.
            wt = per.tile([128, B], BF16, tag="warm", name="wt")
            nc.vector.memset(wt[:], 0.0)
            wps = ps.tile([128, B], F32, tag="ps", name="wps")
            for _ in range(10):
                nc.tensor.matmul(wps[:], wt[:, 0:128], wt[:], start=True, stop=True)

            # ---------------- input loads ----------------
            zv_sb, V_sb = [], []
            for kci, (r0, rows) in enumerate(KCH):
                zt = per.tile([rows, B + 8], F32, tag=f"zv{kci}", name=f"zv{kci}")
                nc.sync.dma_start(zt[:], zv_d[r0:r0 + rows, :])
                zv_sb.append(zt)
                V_sb.append(per.tile([rows, B], BF16, tag=f"V{kci}", name=f"V{kci}"))
            zrep_sb = [t[:, 0:B] for t in zv_sb]
            vcoef_sb = [t[:, B:B + 8] for t in zv_sb]

            zrpad_sb, U_sb, outT_sb = [], [], []
            for bc in range(NBC):
                pt = per.tile([128, M + 32], F32, tag=f"zrpad{bc}", name=f"zrpad{bc}")
                nc.sync.dma_start(pt[:], zrpad_d[bc * 128:(bc + 1) * 128, :])
                zrpad_sb.append(pt)
                U_sb.append(per.tile([128, M], F32, tag=f"U{bc}", name=f"U{bc}"))
                outT_sb.append(per.tile([128, OSH], F32, tag=f"outT{bc}", name=f"outT{bc}"))

            G_sb = []
            for o in range(OSH):
                row = []
                for kci, (r0, rows) in enumerate(KCH):
                    gtile = per.tile([rows, M], BF16, tag=f"G{o}_{kci}", name=f"G{o}_{kci}")
                    nc.sync.dma_start(gtile[:], gt_d[o, r0:r0 + rows, :])
                    row.append(gtile)
                G_sb.append(row)

            # ---------------- build V[(q,iz), b] ----------------
            # rows r = q*32 + iz;  L = (z - bd[q-1])*il[q-1], R = (bd[q+1] - z)*il[q]
            # V = min(relu(L), relu(R)); per-row (scale,bias) from vcoef cols.
            for kci, (r0, rows) in enumerate(KCH):
                lp = tmp.tile([rows, B], F32, tag=f"vL{kci}", name=f"vL{kci}")
                nc.scalar.activation(lp[:], zrep_sb[kci], AF.Relu,
                                     bias=vcoef_sb[kci][:, 1:2], scale=vcoef_sb[kci][:, 0:1])
                rp = tmp.tile([rows, B], F32, tag=f"vR{kci}", name=f"vR{kci}")
                nc.scalar.activation(rp[:], zrep_sb[kci], AF.Relu,
                                     bias=vcoef_sb[kci][:, 3:4], scale=vcoef_sb[kci][:, 2:3])
                nc.vector.tensor_tensor(V_sb[kci][:], lp[:], rp[:], ALU.min)

            # tail fixes: q=1 keeps L un-relu'd (left extrapolation), q=15 keeps R raw.
            fl = tmp.tile([32, B], F32, tag="vfq1L", name="vfixq1L")
            nc.scalar.activation(fl[:], zv_sb[0][32:64, 0:B], AF.Copy,
                                 bias=float(-bd[0] * il[0]), scale=float(il[0]))
            fr = tmp.tile([32, B], F32, tag="vfq1R", name="vfixq1R")
            nc.scalar.activation(fr[:], zv_sb[0][32:64, 0:B], AF.Relu,
                                 bias=zv_sb[0][32:64, B + 4:B + 5], scale=float(-il[1]))
            nc.vector.tensor_tensor(V_sb[0][32:64, :], fl[:], fr[:], ALU.min)

            fl2 = tmp.tile([32, B], F32, tag="vfq15L", name="vfixq15L")
            nc.scalar.activation(fl2[:], zv_sb[3][96:128, 0:B], AF.Relu,
                                 bias=zv_sb[3][96:128, B + 4:B + 5], scale=float(il[14]))
            fr2 = tmp.tile([32, B], F32, tag="vfq15R", name="vfixq15R")
            nc.scalar.activation(fr2[:], zv_sb[3][96:128, 0:B], AF.Copy,
                                 bias=float(bd[16] * il[15]), scale=float(-il[15]))
            nc.vector.tensor_tensor(V_sb[3][96:128, :], fl2[:], fr2[:], ALU.min)

            # ---------------- build U[b, (p,ix)] (transposed layout) ----------------
            # zrpad col 32+p*32+ix = R_p(x[ix,b]); identity L_p = 1 - R_{p-1} gives the
            # L operand as a 32-col-shifted read of the same tensor.
            for bc in range(NBC):
                lp = tmp.tile([128, M], F32, tag=f"uL{bc}", name=f"uL{bc}")
                nc.scalar.activation(lp[:], zrpad_sb[bc][:, 0:M], AF.Relu,
                                     bias=1.0, scale=-1.0)
                rp = tmp.tile([128, M], F32, tag=f"uR{bc}", name=f"uR{bc}")
                nc.scalar.activation(rp[:], zrpad_sb[bc][:, 32:M + 32], AF.Relu)
                nc.vector.tensor_tensor(U_sb[bc][:], lp[:], rp[:], ALU.min)
                # p=1 fix: L un-relu'd
                fx = tmp.tile([128, 32], F32, tag=f"ufix{bc}", name=f"ufix{bc}")
                nc.scalar.activation(fx[:], zrpad_sb[bc][:, 32:64], AF.Copy,
                                     bias=1.0, scale=-1.0)
                nc.vector.tensor_tensor(U_sb[bc][:, 32:64], fx[:], rp[:, 32:64], ALU.min)
                # p=15 fix: R raw (direct from input)
                nc.vector.tensor_tensor(U_sb[bc][:, 480:512], lp[:, 480:512],
                                        zrpad_sb[bc][:, 512:544], ALU.min)

            if debug:
                for kci, (r0, rows) in enumerate(KCH):
                    nc.sync.dma_start(dbgv_d[r0:r0 + rows, :], V_sb[kci][:])
                for bc in range(NBC):
                    nc.sync.dma_start(dbgu_d[bc * 128:(bc + 1) * 128, :], U_sb[bc][:])

            # ---------------- main loop: matmuls + fused reduce ----------------
            for o in range(OSH):
                for bc in range(NBC):
                    psA = ps.tile([128, NSPL], F32, tag="ps", name=f"psA{o}_{bc}")
                    psB = ps.tile([128, NSPL], F32, tag="ps", name=f"psB{o}_{bc}")
                    for kci, (r0, rows) in enumerate(KCH):
                        lhs = V_sb[kci][0:rows, bc * 128:(bc + 1) * 128]
                        st, sp = (kci == 0), (kci == len(KCH) - 1)
                        nc.tensor.matmul(psA[:], lhs,
                                         G_sb[o][kci][0:rows, 0:NSPL],
                                         start=st, stop=sp)
                        nc.tensor.matmul(psB[:], lhs,
                                         G_sb[o][kci][0:rows, NSPL:M],
                                         start=st, stop=sp)
                    if debug and o == 0:
                        dtmpA = sc.tile([128, NSPL], F32, tag="dbg", name=f"dbgA{bc}")
                        nc.vector.tensor_copy(dtmpA[:], psA[:])
                        nc.sync.dma_start(dbgc_d[bc * 128:(bc + 1) * 128, 0:NSPL], dtmpA[:])
                        dtmpB = sc.tile([128, NSPL], F32, tag="dbg", name=f"dbgB{bc}")
                        nc.vector.tensor_copy(dtmpB[:], psB[:])
                        nc.sync.dma_start(dbgc_d[bc * 128:(bc + 1) * 128, NSPL:M], dtmpB[:])
                    scA = sc.tile([128, NSPL], F32, tag="sc", name=f"scA{o}_{bc}")
                    acc1 = ac.tile([128, 1], F32, tag="ac", name=f"acc1_{o}_{bc}")
                    nc.vector.scalar_tensor_tensor(
                        out=scA[:], in0=psA[:], scalar=1.0, in1=U_sb[bc][:, 0:NSPL],
                        op0=ALU.mult, op1=ALU.mult, accum_out=acc1[:])
                    scB = sc.tile([128, NSPL], F32, tag="sc", name=f"scB{o}_{bc}")
                    acc2 = ac.tile([128, 1], F32, tag="ac2", name=f"acc2_{o}_{bc}")
                    nc.vector.scalar_tensor_tensor(
                        out=scB[:], in0=psB[:], scalar=1.0, in1=U_sb[bc][:, NSPL:M],
                        op0=ALU.mult, op1=ALU.mult, accum_out=acc2[:])
                    nc.vector.tensor_add(outT_sb[bc][:, o:o + 1], acc1[:], acc2[:])

            # ---------------- store ----------------
            for bc in range(NBC):
                nc.sync.dma_start(out_d[bc * 128:(bc + 1) * 128, :], outT_sb[bc][:])

    nc.finalize()
    return nc


def _host_prep(x, z, func_parameter, borders, il):
    x = np.asarray(x, np.float32)
    z = np.asarray(z, np.float32)
    F = np.asarray(func_parameter, np.float32)
    bd = np.asarray(borders, np.float32)
    il = np.asarray(il, np.float32)

    # G_all[o, q*32+iz, p*32+ix] = F[p,q,o,ix,iz]
    import ml_dtypes
    G_all = np.ascontiguousarray(
        F.transpose(2, 1, 4, 0, 3)).reshape(OUT, K, M).astype(ml_dtypes.bfloat16)

    zrep = np.tile(z, (NQ, 1))                                # [544, 512]

    q = np.arange(NQ)
    aL = np.where(q >= 1, il[np.clip(q - 1, 0, NG - 1)], 0.0)
    bL = np.where(q >= 1, -bd[np.clip(q - 1, 0, NQ - 1)] * il[np.clip(q - 1, 0, NG - 1)], BIG)
    aR = np.where(q <= NG - 1, -il[np.clip(q, 0, NG - 1)], 0.0)
    bR = np.where(q <= NG - 1, bd[np.clip(q + 1, 0, NQ - 1)] * il[np.clip(q, 0, NG - 1)], BIG)
    fixb = np.zeros(NQ)
    fixb[1] = bd[2] * il[1]          # V q=1 fix: R-side relu bias
    fixb[15] = -bd[14] * il[14]      # V q=15 fix: L-side relu bias
    zero = np.zeros(NQ)
    vcoef = np.ascontiguousarray(
        np.stack([np.repeat(c.astype(np.float32), IZ)
                  for c in (aL, bL, aR, bR, fixb, zero, zero, zero)], axis=1))

    Rx = np.empty((NQ, IX, B), np.float32)
    for p in range(NG):
        Rx[p] = (bd[p + 1] - x) * il[p]
    Rx[NG] = BIG
    ZR_T = Rx.transpose(2, 0, 1).reshape(B, M)
    zrpad = np.ascontiguousarray(
        np.concatenate([np.full((B, 32), -BIG, np.float32), ZR_T], axis=1))

    zv = np.ascontiguousarray(np.concatenate([zrep, vcoef], axis=1))  # [544, 520]
    return G_all, zv, zrpad, bd, il


def kernel(x, z, func_parameter, borders, inverse_chunk_lengths, _trace=False):
    G_all, zv, zrpad, bd, il = _host_prep(
        x, z, func_parameter, borders, inverse_chunk_lengths)

    key = (bd.tobytes(), il.tobytes())
    if key not in _NC_CACHE:
        _NC_CACHE[key] = _build_nc(bd, il)
    nc = _NC_CACHE[key]

    in_maps = []
    for c in range(NCORES):
        in_maps.append({
            "gt": np.ascontiguousarray(G_all[c * OSH:(c + 1) * OSH]),
            "zv": zv,
            "zrpad": zrpad,
        })

    res = run_bass_kernel_spmd(nc, in_maps, core_ids=list(range(NCORES)),
                               trace=_trace)
    out = np.concatenate([res.results[c]["out"].T for c in range(NCORES)], axis=0)
    out = np.ascontiguousarray(out.astype(np.float32))
    if _trace:
        return out, res
    return out
